# revision 28
# baseline (speedup 1.0000x reference)
"""DynamicLinear (MoE routing) Trainium2 Bass kernel.

Math (per sample b):
    out[b] = sum_k attn[b,k] * (x[b] @ W[k].T + bias[k])
           = sum_k attn[b,k] * (x[b] @ W[k].T) + attn[b] @ bias

Sharding: 8 cores in a 2x4 grid over (batch, out_features).
Each core computes out[b_half, o_quarter] from x[b_half] and
W[:, o_quarter, :] -- no cross-core communication.

The host ships x and W pre-tiled and pre-cast to bf16 in the exact
SBUF layouts the kernel consumes; every load is a plain full-rate
HWDGE DMA.  Matmuls run bf16 x bf16 with fp32 PSUM accumulation.

Schedule (evolved over 11 profiled iterations; measured ~243us vs
~248-260us for the b-tile-major baseline):
  - 6 dummy warm-up matmuls on a zeroed scratch tile issue right after
    the Tile prologue (~7.3us) so the PE is busy while the first DMAs
    land (the HAM clock-gate needs ~3.4us of *unbroken* PE activity to
    reach the full 2.4 GHz).
  - Head DMAs strictly need-ordered and alternated between the two
    HWDGE rings (which ring starts first varies run to run): expert-0
    W in 1-2 ii-tile granules interleaved with x tiles 0..2 in pieces,
    then attn.  Host pre-transposes attn and pre-replicates bias --
    gather APs and SWDGE broadcasts starved the head in earlier
    versions.
  - Phase A processes b-tiles 0..2 granule-major (staircase): each
    arriving W0 granule feeds 3-6 real matmuls across three PSUM
    banks, so the PE never idles long enough to re-throttle; thin
    granule batches are padded with dependency-free dummy matmuls.
    HAM reaches 8/8 at ~13us and stays there for the whole kernel.
  - Remaining x tiles: single-tile DMAs for t=3..6 (completion latency
    ~2us on top of byte arrival), then 2-3-tile blocks; bias slotted
    where first needed (~28us); W1 then W2+W3 (one 4 MiB DMA) last.
  - 7 rotating PSUM banks for groups + 1 for warm-up/pads.
  - k=3 combine + store run in [128,256] halves; the very last tile in
    shrinking pieces with stores alternating rings (DMA issue is
    ~0.6us per instruction per engine NX).

Per-core schedule: expert-outer sweeps (k = 0..3); x tiles stay
resident in SBUF after sweep 0.  Per (expert, b_tile): 16 matmul
passes (K=128 contraction, N=512 moving) accumulate in one PSUM bank;
DVE combines acc[t] = sum_k attn[:,k]*(bias[k] + psum_k) with attn as
per-partition scalar; out stores after the last expert.
"""

import numpy as np

_B, _K, _IN, _OUT = 4096, 4, 2048, 2048
_GRID_B, _GRID_O = 2, 4
_BL = _B // _GRID_B      # 2048 batch rows per core
_OL = _OUT // _GRID_O    # 512 out cols per core
_NBT = _BL // 128        # 16 b tiles
_NIT = _IN // 128        # 16 contraction tiles

_CACHE = {}
LAST_RESULTS = None


def _build_program():
    import concourse.bass as bass
    import concourse.tile as tile
    from concourse import bacc, mybir

    f32 = mybir.dt.float32
    bf16 = mybir.dt.bfloat16
    MULT = mybir.AluOpType.mult
    ADD = mybir.AluOpType.add

    nc = bacc.Bacc("TRN2", target_bir_lowering=False, debug=False)
    # host-pretiled layouts: every load is contiguous per partition
    xT = nc.dram_tensor("xT", [_NBT, 128, _NIT, 128], bf16,
                        kind="ExternalInput").ap()
    # attn pre-transposed on host: attn[p, t, k] = attn_orig[t*128+p, k]
    # -- a 16B-granule gather AP here costs ~2048 tiny DMA descriptors
    # that grind the HWDGE ring for ~10us (v2 lesson).
    attn = nc.dram_tensor("attn", [128, _NBT, _K], f32,
                          kind="ExternalInput").ap()
    wT = nc.dram_tensor("wT", [_K, 128, _NIT, _OL], bf16,
                        kind="ExternalInput").ap()
    # bias pre-replicated on host to all 128 partitions: an SWDGE
    # stride-0 broadcast here bursts ~280 GB/s at 10-14us and starves
    # the head-critical W0/x0 loads (v4 lesson).
    biasr = nc.dram_tensor("biasr", [128, _K, _OL], f32,
                           kind="ExternalInput").ap()
    out = nc.dram_tensor("out", [_BL, _OL], f32, kind="ExternalOutput").ap()

    # expert-0 granule sizes in ii-tiles: small head so the first matmul
    # can start as soon as ~1 ii-tile (128 KiB) of W0 has landed
    _G0 = [1, 1, 2, 2, 2, 2, 2, 2, 2]
    _X0 = [2, 6, 8]   # x piece sizes (ii-tiles) for the staircase tiles
    _NSTAIR = 3       # b-tiles processed granule-major in phase A
    _NWARM = 6

    with tile.TileContext(nc) as tc:
        with (
            tc.tile_pool(name="w0", bufs=1) as w0p,
            tc.tile_pool(name="wt", bufs=1) as wtp,
            tc.tile_pool(name="xt", bufs=1) as xtp,
            tc.tile_pool(name="xt0", bufs=1) as xt0p,
            tc.tile_pool(name="singles", bufs=1) as singles,
            tc.tile_pool(name="acc", bufs=_NBT) as accp,
            tc.tile_pool(name="psum", bufs=7, space="PSUM") as psump,
            tc.tile_pool(name="warmps", bufs=1, space="PSUM") as warmpsp,
        ):
            # --- HAM warm-up: zeroed scratch, 6 N=512 matmuls (~2.6us
            # cold) issued before any DMA-dependent work so the PE is at
            # K=8/8 when real data arrives.
            scratch = singles.tile([128, _OL], bf16, name="warm_src")
            nc.vector.memset(scratch, 0)
            warm_ps = warmpsp.tile([128, _OL], f32, name="warm_ps")
            for i in range(_NWARM):
                nc.tensor.matmul(warm_ps, lhsT=scratch[:, 0:128],
                                 rhs=scratch, start=True, stop=True)

            # --- DMA issue order (= per-ring FIFO order) -------------
            # Need-ordered, strictly alternating between the two HWDGE
            # rings (which ring starts first varies run to run).  The
            # head carries W0 + x tiles 0..NSTAIR-1 in ii-granule pieces
            # so phase A below always has ready matmul work.
            w0g = []        # (tile, first_ii, n_ii)
            rings = [nc.scalar, nc.sync]
            ring_i = [0]

            def next_ring():
                eng = rings[ring_i[0]]
                ring_i[0] ^= 1
                return eng

            def load_w0(g, ii0, n):
                t_ = w0p.tile([128, n, _OL], bf16, tag=f"w0g{g}",
                              name=f"w0g{g}")
                next_ring().dma_start(out=t_, in_=wT[0, :, ii0:ii0 + n])
                w0g.append((t_, ii0, n))

            xpc = {}        # (t, ii) -> (tile, offset)

            def load_xp(t, p, ii0, n):
                t_ = xt0p.tile([128, n, 128], bf16, tag=f"x{t}p{p}",
                               name=f"x{t}p{p}")
                next_ring().dma_start(out=t_, in_=xT[t, :, ii0:ii0 + n])
                for j in range(n):
                    xpc[(t, ii0 + j)] = (t_, j)

            _GOFF = [sum(_G0[:g]) for g in range(len(_G0))]
            _XOFF = [sum(_X0[:p]) for p in range(len(_X0))]
            # head, in need order: granule g unlocks NSTAIR tiles' MMs
            load_w0(0, _GOFF[0], _G0[0])
            for t in range(_NSTAIR):
                load_xp(t, 0, _XOFF[0], _X0[0])
            load_w0(1, _GOFF[1], _G0[1])
            load_w0(2, _GOFF[2], _G0[2])
            for t in range(_NSTAIR):
                load_xp(t, 1, _XOFF[1], _X0[1])
            load_w0(3, _GOFF[3], _G0[3])
            load_w0(4, _GOFF[4], _G0[4])
            load_w0(5, _GOFF[5], _G0[5])
            for t in range(_NSTAIR):
                load_xp(t, 2, _XOFF[2], _X0[2])
            for g in range(6, len(_G0)):
                load_w0(g, _GOFF[g], _G0[g])

            w0_of = {}      # ii -> (tile, offset within tile)
            for t_, first, n in w0g:
                for j in range(n):
                    w0_of[first + j] = (t_, j)

            attn_sb = singles.tile([128, _NBT, _K], f32)
            next_ring().dma_start(out=attn_sb, in_=attn)

            def load_x_block(t0, t1):
                # one DMA for x tiles [t0, t1): dst [128, t1-t0, NIT, 128]
                nt = t1 - t0
                blk = xtp.tile([128, nt, _NIT, 128], bf16, tag=f"xb{t0}",
                               name=f"xb{t0}")
                src = bass.AP(
                    tensor=xT.tensor,
                    offset=xT.offset + t0 * 128 * _NIT * 128,
                    ap=[[_NIT * 128, 128], [128 * _NIT * 128, nt],
                        [128, _NIT], [1, 128]],
                )
                next_ring().dma_start(out=blk, in_=src)
                return blk

            # x tiles after the staircase: singles first (completion
            # latency ~2us on top of byte arrival makes early blocks
            # late -- v5/v6 lesson), then growing blocks; bias slotted
            # where it's needed (~30us); W1, W2+W3 last.
            xblk = {}   # t -> (block tile, index within block)
            for t0, t1 in ((3, 4), (4, 5), (5, 6), (6, 7)):
                blk = load_x_block(t0, t1)
                for t in range(t0, t1):
                    xblk[t] = (blk, t - t0)

            bias_rep = singles.tile([128, _K, _OL], f32)
            next_ring().dma_start(out=bias_rep, in_=biasr)

            for t0, t1 in ((7, 9), (9, 11), (11, 13), (13, _NBT)):
                blk = load_x_block(t0, t1)
                for t in range(t0, t1):
                    xblk[t] = (blk, t - t0)

            wt1 = wtp.tile([128, _NIT, _OL], bf16, tag="wt1", name="wt1")
            next_ring().dma_start(out=wt1, in_=wT[1])
            # W2 + W3 as one 4 MiB DMA (fewer completion sems)
            wt23 = wtp.tile([128, 2, _NIT, _OL], bf16, tag="wt23",
                            name="wt23")
            src23 = bass.AP(
                tensor=wT.tensor,
                offset=wT.offset + 2 * 128 * _NIT * _OL,
                ap=[[_NIT * _OL, 128], [128 * _NIT * _OL, 2],
                    [_OL, _NIT], [1, _OL]],
            )
            next_ring().dma_start(out=wt23, in_=src23)

            def rhs_of(k, ii):
                if k == 0:
                    t_, j = w0_of[ii]
                    return t_[:, j, :]
                if k == 1:
                    return wt1[:, ii, :]
                return wt23[:, k - 2, ii, :]

            def lhsT_of(t, ii):
                if t < _NSTAIR:
                    t_, j = xpc[(t, ii)]
                    return t_[:, j, :]
                blk, j = xblk[t]
                return blk[:, j, ii, :]

            # --- phase A: granule-major staircase over the first
            # NSTAIR b-tiles.  Per 2-ii granule this is 2*NSTAIR
            # matmuls (~2.6us cold) vs ~1.6us granule arrival, so the
            # PE stays continuously busy from the first granule and the
            # HAM clock-gate reaches 8/8 by ~14us (v6: cold till 33us).
            acc = [None] * _NBT
            ps_a = [psump.tile([128, _OL], f32, tag="ps", name=f"psA{t}")
                    for t in range(_NSTAIR)]
            # dummy pads after the thin 1-ii granule batches:
            # dependency-free MMs (scratch is already resident) that
            # fill the wait for the next granule so the HAM busy-streak
            # never breaks (v7: two sub-us gaps kept the PE at K=4/8
            # until 20.7us).  With NSTAIR=4 the 2-ii granules carry 8
            # real MMs (~3.4us cold) -- no pads needed there.
            _PADS = {0: 3, 1: 3}
            for g, (t_, first, n) in enumerate(w0g):
                for j in range(n):
                    ii = first + j
                    for t in range(_NSTAIR):
                        nc.tensor.matmul(
                            ps_a[t],
                            lhsT=lhsT_of(t, ii),
                            rhs=t_[:, j, :],
                            start=(ii == 0), stop=(ii == _NIT - 1),
                        )
                for _ in range(_PADS.get(g, 1 if g < 8 else 0)):
                    nc.tensor.matmul(warm_ps, lhsT=scratch[:, 0:128],
                                     rhs=scratch, start=True, stop=True)

            def combine(k, t, ps, a_sc):
                if k == 0:
                    # init acc with the full bias combination (DVE)
                    at = accp.tile([128, _OL], f32, tag="acc",
                                   name=f"acc{t}")
                    acc[t] = at
                    nc.vector.tensor_scalar(
                        out=at, in0=bias_rep[:, 0, :],
                        scalar1=a_sc[:, 0:1], scalar2=None, op0=MULT,
                    )
                    for kk in range(1, _K):
                        nc.vector.scalar_tensor_tensor(
                            out=at, in0=bias_rep[:, kk, :],
                            scalar=a_sc[:, kk:kk + 1], in1=at,
                            op0=MULT, op1=ADD,
                        )
                nc.vector.scalar_tensor_tensor(
                    out=acc[t], in0=ps, scalar=a_sc[:, k:k + 1],
                    in1=acc[t], op0=MULT, op1=ADD,
                )

            for t in range(_NSTAIR):
                combine(0, t, ps_a[t], attn_sb[:, t, :])

            # --- main sweeps (k=0 continues at t=NSTAIR) -------------
            for k in range(_K):
                for t in range(_NSTAIR if k == 0 else 0, _NBT):
                    a_sc = attn_sb[:, t, :]
                    ps = psump.tile([128, _OL], f32, tag="ps",
                                    name=f"ps{k}_{t}")
                    for ii in range(_NIT):
                        nc.tensor.matmul(
                            ps,
                            lhsT=lhsT_of(t, ii),
                            rhs=rhs_of(k, ii),
                            start=(ii == 0), stop=(ii == _NIT - 1),
                        )
                    if k < _K - 1:
                        combine(k, t, ps, a_sc)
                    else:
                        # final sweep: combine + store in pieces so the
                        # kernel tail is one short op + store, not a
                        # full-tile chain.  The very last tile shrinks
                        # its pieces progressively and alternates store
                        # rings (DMA issue is ~0.6us per instruction on
                        # one engine's NX).
                        if t == _NBT - 1:
                            pieces = [256, 128, 64, 64]
                        else:
                            pieces = [256, 256]
                        off = 0
                        for h, w in enumerate(pieces):
                            sl = slice(off, off + w)
                            off += w
                            nc.vector.scalar_tensor_tensor(
                                out=acc[t][:, sl], in0=ps[:, sl],
                                scalar=a_sc[:, k:k + 1],
                                in1=acc[t][:, sl], op0=MULT, op1=ADD,
                            )
                            eng = nc.sync if (t == _NBT - 1 and h % 2) \
                                else nc.scalar
                            eng.dma_start(
                                out=out[t * 128:(t + 1) * 128, sl],
                                in_=acc[t][:, sl],
                            )

    nc.compile()
    return nc


def _get_program():
    if "nc" not in _CACHE:
        _CACHE["nc"] = _build_program()
    return _CACHE["nc"]


def _ensure_axon_hooks_importable():
    """bass_utils' trace branch imports antenv.axon_hooks, which the
    trimmed agent image may lack; stub it (hook=None) so a stray
    BASS_TRACE=1 degrades to an untraced run instead of crashing."""
    import sys
    import types

    try:
        import antenv.axon_hooks  # noqa: F401
        return
    except ImportError:
        pass
    mod = types.ModuleType("antenv.axon_hooks")
    mod._hook = None
    mod.get_axon_ntff_profile_hook = lambda: mod._hook

    def _set(h):
        mod._hook = h

    mod.set_axon_ntff_profile_hook = _set
    sys.modules["antenv.axon_hooks"] = mod
    try:
        import antenv
        antenv.axon_hooks = mod
    except ImportError:
        pass


def kernel(**inputs):
    global LAST_RESULTS
    from concourse.bass_utils import run_bass_kernel_spmd

    _ensure_axon_hooks_importable()

    x = np.ascontiguousarray(inputs["x"], dtype=np.float32)
    attn = np.ascontiguousarray(inputs["softmax_attention"], dtype=np.float32)
    w = np.ascontiguousarray(inputs["weight"], dtype=np.float32)
    b = np.ascontiguousarray(inputs["bias"], dtype=np.float32)

    nc = _get_program()
    in_maps = []
    for c in range(8):
        gb, go = divmod(c, _GRID_O)
        x_sl = x[gb * _BL:(gb + 1) * _BL]
        w_sl = w[:, go * _OL:(go + 1) * _OL, :]
        # tile-contiguous device layouts (see _build_program):
        # xT[t, i_in, ii, b_in] = x[t*128 + b_in, ii*128 + i_in]
        # wT[k, i_in, ii, o]    = W[k, o, ii*128 + i_in]
        import ml_dtypes
        xT = np.ascontiguousarray(
            x_sl.T.reshape(_NIT, 128, _NBT, 128).transpose(2, 1, 0, 3)
        ).astype(ml_dtypes.bfloat16)
        wTa = np.ascontiguousarray(
            w_sl.transpose(0, 2, 1)
            .reshape(_K, _NIT, 128, _OL).transpose(0, 2, 1, 3)
        ).astype(ml_dtypes.bfloat16)
        # attn pre-transposed: attnT[p, t, k] = attn[t*128 + p, k]
        attnT = np.ascontiguousarray(
            attn[gb * _BL:(gb + 1) * _BL]
            .reshape(_NBT, 128, _K).transpose(1, 0, 2)
        )
        # bias replicated to all 128 partitions on host (plain HWDGE
        # load on-device; SWDGE stride-0 broadcast starves the head)
        biasr = np.ascontiguousarray(
            np.broadcast_to(b[None, :, go * _OL:(go + 1) * _OL],
                            (128, _K, _OL))
        )
        in_maps.append({
            "xT": xT,
            "attn": attnT,
            "wT": wTa,
            "biasr": biasr,
        })

    res = run_bass_kernel_spmd(nc, in_maps, list(range(8)))
    LAST_RESULTS = res

    full = np.empty((_B, _OUT), dtype=np.float32)
    for c in range(8):
        gb, go = divmod(c, _GRID_O)
        full[gb * _BL:(gb + 1) * _BL, go * _OL:(go + 1) * _OL] = \
            res.results[c]["out"]
    return full
